# revision 17
# baseline (speedup 1.0000x reference)
"""Bass/Tile kernel for nn_BitDanceFP8ScaledLinear (column-parallel over 8 NeuronCores).

y = x @ (weight * weight_scale[:, None]).T + bias
  x: [4, 2048, 4096] f32, weight: [11008, 4096] f32, weight_scale/bias: [11008] f32

Strategy (per core c of 8):
  - weight/scale/bias sharded along out_features (1376 per core); x replicated.
  - Host-side (lossless layout prep only): x flattened+transposed to xT [4096, 8192];
    weight shard transposed to wT [4096, 1376]; scale/bias replicated to [128, 1376].
  - Device: wT and xT are DMA-loaded with an inline fp32->bf16 cast (SWDGE round-to-
    nearest), matmuls run in bf16 at full PE rate accumulating fp32 in PSUM
    (psum[tokens=128, outF<=512] += xT_tile.T @ wT_tile over 32 k-chunks),
    epilogue applies y = psum * scale + bias on the vector engine (per-column
    vectors pre-replicated across partitions), result DMA'd to y [8192, 1376] f32.
  - Host gathers: concatenate core outputs along out_features.
"""

import sys

for _p in ("/opt/trn_rl_repo", "/root/.axon_site/_ro/trn_rl_repo"):
    if _p not in sys.path:
        sys.path.insert(0, _p)

import numpy as np

import concourse.tile as tile
from concourse.tile import add_dep_helper
from concourse import bacc, bass_utils, mybir

B, S, IN, OUT = 4, 2048, 4096, 11008
N_CORES = 8
OUT_SH = OUT // N_CORES  # 1376
TOKENS = B * S  # 8192
P = 128
KO = IN // P  # 32 contraction chunks
T_BLK = 256  # tokens per x load block
N_SPLITS = [(0, 512), (512, 512), (1024, 352)]  # OUT_SH split into PSUM-bank-sized pieces

_cache = {}


def _build_program():
    nc = bacc.Bacc("TRN2", target_bir_lowering=False, debug=False, num_devices=N_CORES)

    xT = nc.dram_tensor("xT", [IN, TOKENS], mybir.dt.float32, kind="ExternalInput").ap()
    wT = nc.dram_tensor("wT", [IN, OUT_SH], mybir.dt.float32, kind="ExternalInput").ap()
    sc = nc.dram_tensor("scale_rep", [P, OUT_SH], mybir.dt.float32, kind="ExternalInput").ap()
    bi = nc.dram_tensor("bias_rep", [P, OUT_SH], mybir.dt.float32, kind="ExternalInput").ap()
    y = nc.dram_tensor("y", [TOKENS, OUT_SH], mybir.dt.float32, kind="ExternalOutput").ap()

    xT_t = xT.rearrange("(ko ki) t -> ki ko t", ki=P)  # [128, 32, 8192]
    wT_t = wT.rearrange("(ko ki) n -> ki ko n", ki=P)  # [128, 32, 1376]

    NB = T_BLK // P  # m-tiles per block

    with tile.TileContext(nc) as tc:
        with (
            tc.tile_pool(name="const", bufs=1) as const,
            tc.tile_pool(name="wstage", bufs=6) as wstage,
            tc.tile_pool(name="xp", bufs=2) as xp,
            tc.tile_pool(name="outp", bufs=4) as outp,
            tc.tile_pool(name="psum", bufs=8, space="PSUM") as psp,
        ):
            wbk = {}   # (nr, k) -> bf16 tile
            wcast = {}
            xbs = {}   # blk -> bf16 x tile (early blocks)

            def emit_w_range(nr):
                """One third of the weight: 32 HWDGE piece loads + DVE casts."""
                n0, nsz = N_SPLITS[nr]
                for k in range(KO):
                    wst = wstage.tile([P, 512], mybir.dt.float32, name="wst")
                    nc.sync.dma_start(wst[:, :nsz], wT_t[:, k, n0 : n0 + nsz])
                    wbt = const.tile([P, nsz], mybir.dt.bfloat16, name=f"wb_{nr}_{k}")
                    wcast[(nr, k)] = nc.vector.tensor_copy(wbt[:], wst[:, :nsz])
                    wbk[(nr, k)] = wbt

            def emit_x_staged(blk):
                """Early-block x via the SAME Sync/HWDGE FIFO as the weight
                pieces (f32 staged halves + DVE cast): FIFO position is the
                pacing — no SDMA round-robin starvation."""
                t0 = blk * T_BLK
                xb = xp.tile([P, KO, T_BLK], mybir.dt.bfloat16, name="xb")
                H = KO // 2
                for h in range(2):
                    xst = xp.tile([P, H, T_BLK], mybir.dt.float32, name="xstage", bufs=1)
                    nc.sync.dma_start(xst[:], xT_t[:, h * H : (h + 1) * H, t0 : t0 + T_BLK])
                    nc.vector.tensor_copy(xb[:, h * H : (h + 1) * H, :], xst[:])
                xbs[blk] = xb

            # Sync-FIFO program: x0 | w-nr0 | x1 | scale,bias | w-nr1 | x2 | w-nr2
            emit_x_staged(0)
            emit_w_range(0)
            emit_x_staged(1)
            sct = const.tile([P, OUT_SH], mybir.dt.float32)
            nc.sync.dma_start(sct[:], sc[:])
            bit = const.tile([P, OUT_SH], mybir.dt.float32)
            nc.sync.dma_start(bit[:], bi[:])
            emit_w_range(1)
            emit_x_staged(2)
            emit_w_range(2)

            def mm_group(ps, xb, mi, nr):
                """One PSUM accumulation group (mi, nr) over all k."""
                for k in range(KO):
                    nc.tensor.matmul(
                        ps,
                        xb[:, k, mi * P : (mi + 1) * P],
                        wbk[(nr, k)][:],
                        start=(k == 0),
                        stop=(k == KO - 1),
                    )

            def evict(ps, ot, nr):
                n0, nsz = N_SPLITS[nr]
                nc.vector.tensor_mul(ot[:, n0 : n0 + nsz], ps, sct[:, n0 : n0 + nsz])
                nc.vector.tensor_add(
                    ot[:, n0 : n0 + nsz], ot[:, n0 : n0 + nsz], bit[:, n0 : n0 + nsz]
                )

            for blk in range(TOKENS // T_BLK):
                t0 = blk * T_BLK
                if blk in xbs:
                    xb = xbs[blk]
                else:
                    xb = xp.tile([P, KO, T_BLK], mybir.dt.bfloat16, name="xb")
                    xdma = nc.gpsimd.dma_start(xb[:], xT_t[:, :, t0 : t0 + T_BLK])
                    gate = {3: (1, 0), 4: (2, 0)}.get(blk)
                    if gate is not None:
                        add_dep_helper(xdma.ins, wcast[gate].ins, sync=True,
                                       reason="pace x prefetch behind w stream")

                if blk <= 3:
                    # n-range-major group order, k-interleaved across the two
                    # m-tiles: PE work trails the streaming weight pieces.
                    ots = [outp.tile([P, OUT_SH], mybir.dt.float32, name="ot") for _ in range(NB)]
                    for nr in range(len(N_SPLITS)):
                        nsz = N_SPLITS[nr][1]
                        pss = [psp.tile([P, 512], mybir.dt.float32, name="ps")[:, :nsz] for _ in range(NB)]
                        for k in range(KO):
                            for mi in range(NB):
                                nc.tensor.matmul(
                                    pss[mi],
                                    xb[:, k, mi * P : (mi + 1) * P],
                                    wbk[(nr, k)][:],
                                    start=(k == 0),
                                    stop=(k == KO - 1),
                                )
                        for mi in range(NB):
                            evict(pss[mi], ots[mi], nr)
                    for mi in range(NB):
                        trow = t0 + mi * P
                        nc.scalar.dma_start(y[trow : trow + P, :], ots[mi][:])
                    continue

                for mi in range(NB):
                    ot = outp.tile([P, OUT_SH], mybir.dt.float32, name="ot")
                    for nr in range(len(N_SPLITS)):
                        nsz = N_SPLITS[nr][1]
                        ps_full = psp.tile([P, 512], mybir.dt.float32, name="ps")
                        ps = ps_full[:, :nsz]
                        mm_group(ps, xb, mi, nr)
                        evict(ps, ot, nr)
                    trow = t0 + mi * P
                    nc.scalar.dma_start(y[trow : trow + P, :], ot[:])

    nc.compile()
    return nc


def _prep_inputs(x, weight, weight_scale, bias):
    x2 = np.ascontiguousarray(x, dtype=np.float32).reshape(TOKENS, IN)
    xT = np.ascontiguousarray(x2.T)  # [4096, 8192], shared across cores
    in_maps = []
    for c in range(N_CORES):
        lo, hi = c * OUT_SH, (c + 1) * OUT_SH
        wTc = np.ascontiguousarray(weight[lo:hi, :].astype(np.float32, copy=False).T)
        scc = np.ascontiguousarray(
            np.broadcast_to(weight_scale[lo:hi].astype(np.float32, copy=False)[None, :], (P, OUT_SH))
        )
        bic = np.ascontiguousarray(
            np.broadcast_to(bias[lo:hi].astype(np.float32, copy=False)[None, :], (P, OUT_SH))
        )
        in_maps.append({"xT": xT, "wT": wTc, "scale_rep": scc, "bias_rep": bic})
    return in_maps


def kernel(x, weight, weight_scale, bias, _trace=False):
    if "nc" not in _cache:
        _cache["nc"] = _build_program()
    nc = _cache["nc"]
    in_maps = _prep_inputs(x, weight, weight_scale, bias)
    res = bass_utils.run_bass_kernel_spmd(
        nc, in_maps, core_ids=list(range(N_CORES)), trace=_trace
    )
    _cache["last_result"] = res
    out = np.concatenate([res.results[c]["y"] for c in range(N_CORES)], axis=1)
    return out.reshape(B, S, OUT)


# revision 18
# speedup vs baseline: 1.0397x; 1.0397x over previous
"""Bass/Tile kernel for nn_BitDanceFP8ScaledLinear (column-parallel over 8 NeuronCores).

y = x @ (weight * weight_scale[:, None]).T + bias
  x: [4, 2048, 4096] f32, weight: [11008, 4096] f32, weight_scale/bias: [11008] f32

Strategy (per core c of 8):
  - weight/scale/bias sharded along out_features (1376 per core); x replicated.
  - Host-side (lossless layout prep only): x is laid out k-major per 256-token
    block as [32 blocks, 128, 32 kchunks, 256 tokens] so every x-block DMA has
    32KB-contiguous per-partition runs; weight shard transposed to wT
    [4096, 1376]; scale/bias replicated to [128, 1376].
  - Device: x blocks are DMA-loaded with an inline fp32->bf16 cast (SWDGE,
    round-to-nearest). The weight streams n-range-major in 96 [128, nsz] f32
    pieces on the Sync HWDGE FIFO, DVE-cast to persistent bf16 tiles.
    Matmuls run bf16 at full PE rate, accumulating fp32 in PSUM
    (psum[tokens=128, outF<=512] += x_tile.T @ w_piece over 32 k-chunks).
  - Startup coverage: the first 4 blocks' groups run k-interleaved 8-wide
    (4 blocks x 2 m-tiles) at each n-range, so the PE consumes each weight
    piece (~1.7us of matmul) faster than it streams (~1.1us): the PE trails
    the stream with no idle instead of stalling on the 63us weight load.
  - Epilogue per PSUM group: y_piece = psum * scale + bias on DVE (per-column
    vectors pre-replicated across partitions), stored via the ScalarE HWDGE
    queue (separate ring from the weight stream - no head-of-line blocking).
  - Host gathers: concatenate core outputs along out_features.
"""

import sys

for _p in ("/opt/trn_rl_repo", "/root/.axon_site/_ro/trn_rl_repo"):
    if _p not in sys.path:
        sys.path.insert(0, _p)

import numpy as np

import concourse.tile as tile
from concourse.tile import add_dep_helper
from concourse import bacc, bass_utils, mybir

B, S, IN, OUT = 4, 2048, 4096, 11008
N_CORES = 8
OUT_SH = OUT // N_CORES  # 1376
TOKENS = B * S  # 8192
P = 128
KO = IN // P  # 32 contraction chunks
T_BLK = 256  # tokens per x block
NBLK = TOKENS // T_BLK  # 32
NB = T_BLK // P  # m-tiles per block (2)
N_SPLITS = [(0, 512), (512, 512), (1024, 352)]  # OUT_SH split into PSUM-bank-sized pieces
EARLY = 4  # blocks covered by the startup interleave

_cache = {}


def _build_program():
    nc = bacc.Bacc("TRN2", target_bir_lowering=False, debug=False, num_devices=N_CORES)

    xq = nc.dram_tensor("xq", [NBLK, P, KO, T_BLK], mybir.dt.float32, kind="ExternalInput").ap()
    wT = nc.dram_tensor("wT", [IN, OUT_SH], mybir.dt.float32, kind="ExternalInput").ap()
    sc = nc.dram_tensor("scale_rep", [P, OUT_SH], mybir.dt.float32, kind="ExternalInput").ap()
    bi = nc.dram_tensor("bias_rep", [P, OUT_SH], mybir.dt.float32, kind="ExternalInput").ap()
    y = nc.dram_tensor("y", [TOKENS, OUT_SH], mybir.dt.float32, kind="ExternalOutput").ap()

    wT_t = wT.rearrange("(ko ki) n -> ki ko n", ki=P)  # [128, 32, 1376]

    with tile.TileContext(nc) as tc:
        with (
            tc.tile_pool(name="const", bufs=1) as const,
            tc.tile_pool(name="wstage", bufs=6) as wstage,
            tc.tile_pool(name="xp", bufs=EARLY + 1) as xp,
            tc.tile_pool(name="outp", bufs=6) as outp,
            tc.tile_pool(name="psum", bufs=8, space="PSUM") as psp,
        ):
            # x blocks 0..EARLY-1 go out first on the otherwise-empty SWDGE
            # queue (fp32->bf16 cast inline, 32KB-contiguous source runs).
            xbs = {}
            for blk in range(EARLY):
                xb = xp.tile([P, KO, T_BLK], mybir.dt.bfloat16, name="xb")
                nc.gpsimd.dma_start(xb[:], xq[blk])
                xbs[blk] = xb

            # Weight: n-range-major stream of 96 pieces on the Sync HWDGE
            # FIFO, staged f32 then DVE-cast into persistent bf16 tiles.
            wbk = {}
            wcast = {}

            def emit_w_range(nr):
                n0, nsz = N_SPLITS[nr]
                for k in range(KO):
                    wst = wstage.tile([P, 512], mybir.dt.float32, name="wst")
                    nc.sync.dma_start(wst[:, :nsz], wT_t[:, k, n0 : n0 + nsz])
                    wbt = const.tile([P, nsz], mybir.dt.bfloat16, name=f"wb_{nr}_{k}")
                    wcast[(nr, k)] = nc.vector.tensor_copy(wbt[:], wst[:, :nsz])
                    wbk[(nr, k)] = wbt

            emit_w_range(0)
            sct = const.tile([P, OUT_SH], mybir.dt.float32)
            nc.sync.dma_start(sct[:], sc[:])
            bit = const.tile([P, OUT_SH], mybir.dt.float32)
            nc.sync.dma_start(bit[:], bi[:])
            emit_w_range(1)
            emit_w_range(2)

            def evict_store(ps, blk, mi, nr):
                """y_piece = psum * scale + bias; store via ScalarE HWDGE."""
                n0, nsz = N_SPLITS[nr]
                op = outp.tile([P, 512], mybir.dt.float32, name="op")[:, :nsz]
                nc.vector.tensor_mul(op, ps, sct[:, n0 : n0 + nsz])
                nc.vector.tensor_add(op, op, bit[:, n0 : n0 + nsz])
                trow = blk * T_BLK + mi * P
                nc.scalar.dma_start(y[trow : trow + P, n0 : n0 + nsz], op)

            # ---- startup phase: blocks 0..EARLY-1, nr-major, 8-wide k-interleave
            for nr in range(len(N_SPLITS)):
                nsz = N_SPLITS[nr][1]
                groups = [(blk, mi) for blk in range(EARLY) for mi in range(NB)]
                pss = [psp.tile([P, 512], mybir.dt.float32, name="ps")[:, :nsz] for _ in groups]
                for k in range(KO):
                    for g, (blk, mi) in enumerate(groups):
                        nc.tensor.matmul(
                            pss[g],
                            xbs[blk][:, k, mi * P : (mi + 1) * P],
                            wbk[(nr, k)][:],
                            start=(k == 0),
                            stop=(k == KO - 1),
                        )
                for g, (blk, mi) in enumerate(groups):
                    evict_store(pss[g], blk, mi, nr)

            # ---- steady state: blocks EARLY..NBLK-1
            for blk in range(EARLY, NBLK):
                xb = xp.tile([P, KO, T_BLK], mybir.dt.bfloat16, name="xb")
                nc.gpsimd.dma_start(xb[:], xq[blk])
                for mi in range(NB):
                    for nr in range(len(N_SPLITS)):
                        nsz = N_SPLITS[nr][1]
                        ps = psp.tile([P, 512], mybir.dt.float32, name="ps")[:, :nsz]
                        for k in range(KO):
                            nc.tensor.matmul(
                                ps,
                                xb[:, k, mi * P : (mi + 1) * P],
                                wbk[(nr, k)][:],
                                start=(k == 0),
                                stop=(k == KO - 1),
                            )
                        evict_store(ps, blk, mi, nr)

    nc.compile()
    return nc


def _prep_inputs(x, weight, weight_scale, bias):
    x2 = np.ascontiguousarray(x, dtype=np.float32).reshape(TOKENS, IN)
    # [blk, ki, ko, t]: xq[b, ki, ko, t] = x[b*T_BLK + t, ko*P + ki]
    xq = np.ascontiguousarray(
        x2.reshape(NBLK, T_BLK, KO, P).transpose(0, 3, 2, 1)
    )
    in_maps = []
    for c in range(N_CORES):
        lo, hi = c * OUT_SH, (c + 1) * OUT_SH
        wTc = np.ascontiguousarray(weight[lo:hi, :].astype(np.float32, copy=False).T)
        scc = np.ascontiguousarray(
            np.broadcast_to(weight_scale[lo:hi].astype(np.float32, copy=False)[None, :], (P, OUT_SH))
        )
        bic = np.ascontiguousarray(
            np.broadcast_to(bias[lo:hi].astype(np.float32, copy=False)[None, :], (P, OUT_SH))
        )
        in_maps.append({"xq": xq, "wT": wTc, "scale_rep": scc, "bias_rep": bic})
    return in_maps


def kernel(x, weight, weight_scale, bias, _trace=False):
    if "nc" not in _cache:
        _cache["nc"] = _build_program()
    nc = _cache["nc"]
    in_maps = _prep_inputs(x, weight, weight_scale, bias)
    res = bass_utils.run_bass_kernel_spmd(
        nc, in_maps, core_ids=list(range(N_CORES)), trace=_trace
    )
    _cache["last_result"] = res
    out = np.concatenate([res.results[c]["y"] for c in range(N_CORES)], axis=1)
    return out.reshape(B, S, OUT)


# revision 21
# speedup vs baseline: 1.0437x; 1.0038x over previous
"""Bass/Tile kernel for nn_BitDanceFP8ScaledLinear (column-parallel over 8 NeuronCores).

y = x @ (weight * weight_scale[:, None]).T + bias
  x: [4, 2048, 4096] f32, weight: [11008, 4096] f32, weight_scale/bias: [11008] f32

Strategy (per core c of 8):
  - weight/scale/bias sharded along out_features (1376 per core); x replicated.
  - Host-side (lossless layout prep only): x is laid out k-major per 256-token
    block as [32 blocks, 128, 32 kchunks, 256 tokens] so every x-block DMA has
    32KB-contiguous per-partition runs; weight shard transposed to wT
    [4096, 1376]; scale/bias replicated to [128, 1376].
  - Device: x blocks are DMA-loaded with an inline fp32->bf16 cast (SWDGE,
    round-to-nearest). The weight streams n-range-major in 96 [128, nsz] f32
    pieces on the Sync HWDGE FIFO, DVE-cast to persistent bf16 tiles.
    Matmuls run bf16 at full PE rate, accumulating fp32 in PSUM
    (psum[tokens=128, outF<=512] += x_tile.T @ w_piece over 32 k-chunks).
  - Startup coverage: the first 4 blocks' groups run k-interleaved 8-wide
    (4 blocks x 2 m-tiles) at each n-range, so the PE consumes each weight
    piece (~1.7us of matmul) faster than it streams (~1.1us): the PE trails
    the stream with no idle instead of stalling on the 63us weight load.
  - Epilogue per PSUM group: y_piece = psum * scale + bias on DVE (per-column
    vectors pre-replicated across partitions), stored via the ScalarE HWDGE
    queue (separate ring from the weight stream - no head-of-line blocking).
  - Host gathers: concatenate core outputs along out_features.
"""

import sys

for _p in ("/opt/trn_rl_repo", "/root/.axon_site/_ro/trn_rl_repo"):
    if _p not in sys.path:
        sys.path.insert(0, _p)

import numpy as np

import concourse.tile as tile
from concourse.tile import add_dep_helper
from concourse import bacc, bass_utils, mybir

B, S, IN, OUT = 4, 2048, 4096, 11008
N_CORES = 8
OUT_SH = OUT // N_CORES  # 1376
TOKENS = B * S  # 8192
P = 128
KO = IN // P  # 32 contraction chunks
T_BLK = 256  # tokens per x block
NBLK = TOKENS // T_BLK  # 32
NB = T_BLK // P  # m-tiles per block (2)
N_SPLITS = [(0, 512), (512, 512), (1024, 352)]  # OUT_SH split into PSUM-bank-sized pieces
EARLY = 4  # blocks covered by the startup interleave

_cache = {}


def _build_program():
    nc = bacc.Bacc("TRN2", target_bir_lowering=False, debug=False, num_devices=N_CORES)

    xq = nc.dram_tensor("xq", [NBLK, P, KO, T_BLK], mybir.dt.float32, kind="ExternalInput").ap()
    wT = nc.dram_tensor("wT", [IN, OUT_SH], mybir.dt.float32, kind="ExternalInput").ap()
    sc = nc.dram_tensor("scale_rep", [P, OUT_SH], mybir.dt.float32, kind="ExternalInput").ap()
    bi = nc.dram_tensor("bias_rep", [P, OUT_SH], mybir.dt.float32, kind="ExternalInput").ap()
    y = nc.dram_tensor("y", [TOKENS, OUT_SH], mybir.dt.float32, kind="ExternalOutput").ap()

    wT_t = wT.rearrange("(ko ki) n -> ki ko n", ki=P)  # [128, 32, 1376]

    with tile.TileContext(nc) as tc:
        with (
            tc.tile_pool(name="const", bufs=1) as const,
            tc.tile_pool(name="wstage", bufs=6) as wstage,
            tc.tile_pool(name="xp", bufs=2) as xp,
            tc.tile_pool(name="outp", bufs=6) as outp,
            tc.tile_pool(name="psum", bufs=8, space="PSUM") as psp,
        ):
            # Blocks 0-1 arrive as interleaved quarter-tiles on the otherwise
            # empty SWDGE queue (fp32->bf16 cast inline, 8KB-contiguous runs):
            # fine-grained deps let the PE start at the first w piece, and the
            # total x-early bytes (8MB) spread over the w stream keep the
            # piece-arrival rate at ~the PE consumption rate.
            QK = KO // 4  # k-chunks per quarter
            xquart = {}  # (blk, q) -> tile
            for q in range(4):
                for blk in range(2):
                    xt = xp.tile([P, QK, T_BLK], mybir.dt.bfloat16, name=f"xq_{blk}_{q}", bufs=1)
                    nc.gpsimd.dma_start(xt[:], xq[blk, :, q * QK : (q + 1) * QK, :])
                    xquart[(blk, q)] = xt

            def xslice(blk, k, mi):
                if blk < 2:
                    return xquart[(blk, k // QK)][:, k % QK, mi * P : (mi + 1) * P]
                return xbs[blk][:, k, mi * P : (mi + 1) * P]

            xbs = {}

            # Weight: n-range-major stream of 96 pieces on the Sync HWDGE
            # FIFO, staged f32 then DVE-cast into persistent bf16 tiles.
            wbk = {}
            wcast = {}

            def emit_w_range(nr):
                n0, nsz = N_SPLITS[nr]
                for k in range(KO):
                    wst = wstage.tile([P, 512], mybir.dt.float32, name="wst")
                    nc.sync.dma_start(wst[:, :nsz], wT_t[:, k, n0 : n0 + nsz])
                    wbt = const.tile([P, nsz], mybir.dt.bfloat16, name=f"wb_{nr}_{k}")
                    wcast[(nr, k)] = nc.vector.tensor_copy(wbt[:], wst[:, :nsz])
                    wbk[(nr, k)] = wbt

            emit_w_range(0)
            sct = const.tile([P, OUT_SH], mybir.dt.float32)
            nc.sync.dma_start(sct[:], sc[:])
            bit = const.tile([P, OUT_SH], mybir.dt.float32)
            nc.sync.dma_start(bit[:], bi[:])
            emit_w_range(1)
            emit_w_range(2)

            def evict_store(ps, blk, mi, nr):
                """y_piece = psum * scale + bias; store via ScalarE HWDGE."""
                n0, nsz = N_SPLITS[nr]
                op = outp.tile([P, 512], mybir.dt.float32, name="op")[:, :nsz]
                nc.vector.tensor_mul(op, ps, sct[:, n0 : n0 + nsz])
                nc.vector.tensor_add(op, op, bit[:, n0 : n0 + nsz])
                trow = blk * T_BLK + mi * P
                nc.scalar.dma_start(y[trow : trow + P, n0 : n0 + nsz], op)

            # ---- startup phase: blocks 0-1, nr-major, 4-wide k-interleave
            # trailing the weight stream (~0.86us of matmul per ~0.87us piece).
            for nr in range(len(N_SPLITS)):
                nsz = N_SPLITS[nr][1]
                groups = [(blk, mi) for blk in range(2) for mi in range(NB)]
                pss = [psp.tile([P, 512], mybir.dt.float32, name="ps")[:, :nsz] for _ in groups]
                for k in range(KO):
                    for g, (blk, mi) in enumerate(groups):
                        nc.tensor.matmul(
                            pss[g],
                            xslice(blk, k, mi),
                            wbk[(nr, k)][:],
                            start=(k == 0),
                            stop=(k == KO - 1),
                        )
                for g, (blk, mi) in enumerate(groups):
                    evict_store(pss[g], blk, mi, nr)

            # ---- steady state: blocks 2..NBLK-1
            for blk in range(2, NBLK):
                xb = xp.tile([P, KO, T_BLK], mybir.dt.bfloat16, name="xb")
                xdma = nc.gpsimd.dma_start(xb[:], xq[blk])
                gate = {2: (2, 0), 3: (2, 16)}.get(blk)
                if gate is not None:
                    add_dep_helper(xdma.ins, wcast[gate].ins, sync=True,
                                   reason="pace x prefetch behind w stream")
                for mi in range(NB):
                    for nr in range(len(N_SPLITS)):
                        nsz = N_SPLITS[nr][1]
                        ps = psp.tile([P, 512], mybir.dt.float32, name="ps")[:, :nsz]
                        for k in range(KO):
                            nc.tensor.matmul(
                                ps,
                                xb[:, k, mi * P : (mi + 1) * P],
                                wbk[(nr, k)][:],
                                start=(k == 0),
                                stop=(k == KO - 1),
                            )
                        evict_store(ps, blk, mi, nr)

    nc.compile()
    return nc


def _prep_inputs(x, weight, weight_scale, bias):
    x2 = np.ascontiguousarray(x, dtype=np.float32).reshape(TOKENS, IN)
    # [blk, ki, ko, t]: xq[b, ki, ko, t] = x[b*T_BLK + t, ko*P + ki]
    xq = np.ascontiguousarray(
        x2.reshape(NBLK, T_BLK, KO, P).transpose(0, 3, 2, 1)
    )
    in_maps = []
    for c in range(N_CORES):
        lo, hi = c * OUT_SH, (c + 1) * OUT_SH
        wTc = np.ascontiguousarray(weight[lo:hi, :].astype(np.float32, copy=False).T)
        scc = np.ascontiguousarray(
            np.broadcast_to(weight_scale[lo:hi].astype(np.float32, copy=False)[None, :], (P, OUT_SH))
        )
        bic = np.ascontiguousarray(
            np.broadcast_to(bias[lo:hi].astype(np.float32, copy=False)[None, :], (P, OUT_SH))
        )
        in_maps.append({"xq": xq, "wT": wTc, "scale_rep": scc, "bias_rep": bic})
    return in_maps


def kernel(x, weight, weight_scale, bias, _trace=False):
    if "nc" not in _cache:
        _cache["nc"] = _build_program()
    nc = _cache["nc"]
    in_maps = _prep_inputs(x, weight, weight_scale, bias)
    res = bass_utils.run_bass_kernel_spmd(
        nc, in_maps, core_ids=list(range(N_CORES)), trace=_trace
    )
    _cache["last_result"] = res
    out = np.concatenate([res.results[c]["y"] for c in range(N_CORES)], axis=1)
    return out.reshape(B, S, OUT)


# revision 22
# speedup vs baseline: 1.0577x; 1.0135x over previous
"""Bass/Tile kernel for nn_BitDanceFP8ScaledLinear (column-parallel over 8 NeuronCores).

y = x @ (weight * weight_scale[:, None]).T + bias
  x: [4, 2048, 4096] f32, weight: [11008, 4096] f32, weight_scale/bias: [11008] f32

Strategy (per core c of 8):
  - weight/scale/bias sharded along out_features (1376 per core); x replicated.
  - Host-side (lossless layout prep only): x is laid out k-major per 256-token
    block as [32 blocks, 128, 32 kchunks, 256 tokens] so every x-block DMA has
    32KB-contiguous per-partition runs; weight shard transposed to wT
    [4096, 1376]; scale/bias replicated to [128, 1376].
  - Device: x blocks are DMA-loaded with an inline fp32->bf16 cast (SWDGE,
    round-to-nearest). The weight streams n-range-major in 96 [128, nsz] f32
    pieces on the Sync HWDGE FIFO, DVE-cast to persistent bf16 tiles.
    Matmuls run bf16 at full PE rate, accumulating fp32 in PSUM
    (psum[tokens=128, outF<=512] += x_tile.T @ w_piece over 32 k-chunks).
  - Startup coverage: the first 4 blocks' groups run k-interleaved 8-wide
    (4 blocks x 2 m-tiles) at each n-range, so the PE consumes each weight
    piece (~1.7us of matmul) faster than it streams (~1.1us): the PE trails
    the stream with no idle instead of stalling on the 63us weight load.
  - Epilogue per PSUM group: y_piece = psum * scale + bias on DVE (per-column
    vectors pre-replicated across partitions), stored via the ScalarE HWDGE
    queue (separate ring from the weight stream - no head-of-line blocking).
  - Host gathers: concatenate core outputs along out_features.
"""

import sys

for _p in ("/opt/trn_rl_repo", "/root/.axon_site/_ro/trn_rl_repo"):
    if _p not in sys.path:
        sys.path.insert(0, _p)

import numpy as np

import concourse.tile as tile
from concourse.tile import add_dep_helper
from concourse import bacc, bass_utils, mybir

B, S, IN, OUT = 4, 2048, 4096, 11008
N_CORES = 8
OUT_SH = OUT // N_CORES  # 1376
TOKENS = B * S  # 8192
P = 128
KO = IN // P  # 32 contraction chunks
T_BLK = 256  # tokens per x block
NBLK = TOKENS // T_BLK  # 32
NB = T_BLK // P  # m-tiles per block (2)
N_SPLITS = [(0, 512), (512, 512), (1024, 352)]  # OUT_SH split into PSUM-bank-sized pieces
EARLY = 4  # blocks covered by the startup interleave

_cache = {}


def _build_program():
    nc = bacc.Bacc("TRN2", target_bir_lowering=False, debug=False, num_devices=N_CORES)

    xq = nc.dram_tensor("xq", [NBLK, P, KO, T_BLK], mybir.dt.float32, kind="ExternalInput").ap()
    wT = nc.dram_tensor("wT", [IN, OUT_SH], mybir.dt.float32, kind="ExternalInput").ap()
    sc = nc.dram_tensor("scale_rep", [P, OUT_SH], mybir.dt.float32, kind="ExternalInput").ap()
    bi = nc.dram_tensor("bias_rep", [P, OUT_SH], mybir.dt.float32, kind="ExternalInput").ap()
    y = nc.dram_tensor("y", [TOKENS, OUT_SH], mybir.dt.float32, kind="ExternalOutput").ap()

    wT_t = wT.rearrange("(ko ki) n -> ki ko n", ki=P)  # [128, 32, 1376]

    with tile.TileContext(nc) as tc:
        with (
            tc.tile_pool(name="const", bufs=1) as const,
            tc.tile_pool(name="wstage", bufs=6) as wstage,
            tc.tile_pool(name="xp", bufs=2) as xp,
            tc.tile_pool(name="outp", bufs=6) as outp,
            tc.tile_pool(name="psum", bufs=8, space="PSUM") as psp,
        ):
            # Blocks 0-1 arrive as interleaved quarter-tiles on the otherwise
            # empty SWDGE queue (fp32->bf16 cast inline, 8KB-contiguous runs):
            # fine-grained deps let the PE start at the first w piece, and the
            # total x-early bytes (8MB) spread over the w stream keep the
            # piece-arrival rate at ~the PE consumption rate.
            QK = KO // 4  # k-chunks per quarter
            xquart = {}  # (blk, q) -> tile
            for q in range(4):
                for blk in range(2):
                    xt = xp.tile([P, QK, T_BLK], mybir.dt.bfloat16, name=f"xq_{blk}_{q}", bufs=1)
                    nc.gpsimd.dma_start(xt[:], xq[blk, :, q * QK : (q + 1) * QK, :])
                    xquart[(blk, q)] = xt

            def xslice(blk, k, mi):
                if blk < 2:
                    return xquart[(blk, k // QK)][:, k % QK, mi * P : (mi + 1) * P]
                return xbs[blk][:, k, mi * P : (mi + 1) * P]

            xbs = {}

            # Weight: n-range-major stream of 96 pieces on the Sync HWDGE
            # FIFO, staged f32 then DVE-cast into persistent bf16 tiles.
            wbk = {}
            wcast = {}

            def emit_w_range(nr):
                n0, nsz = N_SPLITS[nr]
                for k in range(KO):
                    wst = wstage.tile([P, 512], mybir.dt.float32, name="wst")
                    nc.sync.dma_start(wst[:, :nsz], wT_t[:, k, n0 : n0 + nsz])
                    wbt = const.tile([P, nsz], mybir.dt.bfloat16, name=f"wb_{nr}_{k}")
                    wcast[(nr, k)] = nc.vector.tensor_copy(wbt[:], wst[:, :nsz])
                    wbk[(nr, k)] = wbt

            emit_w_range(0)
            sct = const.tile([P, OUT_SH], mybir.dt.float32)
            nc.sync.dma_start(sct[:], sc[:])
            bit = const.tile([P, OUT_SH], mybir.dt.float32)
            nc.sync.dma_start(bit[:], bi[:])
            emit_w_range(1)
            emit_w_range(2)

            def evict_store(ps, blk, mi, nr):
                """y_piece = psum * scale + bias; store via ScalarE HWDGE."""
                n0, nsz = N_SPLITS[nr]
                op = outp.tile([P, 512], mybir.dt.float32, name="op")[:, :nsz]
                nc.vector.tensor_mul(op, ps, sct[:, n0 : n0 + nsz])
                nc.vector.tensor_add(op, op, bit[:, n0 : n0 + nsz])
                trow = blk * T_BLK + mi * P
                nc.scalar.dma_start(y[trow : trow + P, n0 : n0 + nsz], op)

            # x blocks 2-3: full-tile SWDGE loads gated into nr1's surplus
            # window (the 4-wide nr1 interleave over-covers the stream 2.7x,
            # so their bandwidth theft is absorbed there).
            for blk in (2, 3):
                xb = xp.tile([P, KO, T_BLK], mybir.dt.bfloat16, name="xb")
                xdma = nc.gpsimd.dma_start(xb[:], xq[blk])
                gate = {2: (1, 4), 3: (1, 20)}[blk]
                add_dep_helper(xdma.ins, wcast[gate].ins, sync=True,
                               reason="pace x prefetch behind w stream")
                xbs[blk] = xb

            def xsl(blk, k, mi):
                if blk < 2:
                    return xquart[(blk, k // QK)][:, k % QK, mi * P : (mi + 1) * P]
                return xbs[blk][:, k, mi * P : (mi + 1) * P]

            # ---- startup phase: nr-major k-interleave trailing the stream.
            # nr0/nr1: blocks 0-1 (4 groups); nr2: blocks 0-3 (8 groups).
            for nr in range(len(N_SPLITS)):
                nsz = N_SPLITS[nr][1]
                nblk = 4 if nr == 2 else 2
                groups = [(blk, mi) for blk in range(nblk) for mi in range(NB)]
                pss = [psp.tile([P, 512], mybir.dt.float32, name="ps")[:, :nsz] for _ in groups]
                for k in range(KO):
                    for g, (blk, mi) in enumerate(groups):
                        nc.tensor.matmul(
                            pss[g],
                            xsl(blk, k, mi),
                            wbk[(nr, k)][:],
                            start=(k == 0),
                            stop=(k == KO - 1),
                        )
                for g, (blk, mi) in enumerate(groups):
                    evict_store(pss[g], blk, mi, nr)

            # blocks 2-3: remaining n-ranges (nr0, nr1) dense.
            for blk in (2, 3):
                for mi in range(NB):
                    for nr in (0, 1):
                        nsz = N_SPLITS[nr][1]
                        ps = psp.tile([P, 512], mybir.dt.float32, name="ps")[:, :nsz]
                        for k in range(KO):
                            nc.tensor.matmul(
                                ps,
                                xbs[blk][:, k, mi * P : (mi + 1) * P],
                                wbk[(nr, k)][:],
                                start=(k == 0),
                                stop=(k == KO - 1),
                            )
                        evict_store(ps, blk, mi, nr)

            # ---- steady state: blocks 4..NBLK-1
            for blk in range(4, NBLK):
                xb = xp.tile([P, KO, T_BLK], mybir.dt.bfloat16, name="xb")
                nc.gpsimd.dma_start(xb[:], xq[blk])
                for mi in range(NB):
                    for nr in range(len(N_SPLITS)):
                        nsz = N_SPLITS[nr][1]
                        ps = psp.tile([P, 512], mybir.dt.float32, name="ps")[:, :nsz]
                        for k in range(KO):
                            nc.tensor.matmul(
                                ps,
                                xb[:, k, mi * P : (mi + 1) * P],
                                wbk[(nr, k)][:],
                                start=(k == 0),
                                stop=(k == KO - 1),
                            )
                        evict_store(ps, blk, mi, nr)

    nc.compile()
    return nc


def _prep_inputs(x, weight, weight_scale, bias):
    x2 = np.ascontiguousarray(x, dtype=np.float32).reshape(TOKENS, IN)
    # [blk, ki, ko, t]: xq[b, ki, ko, t] = x[b*T_BLK + t, ko*P + ki]
    xq = np.ascontiguousarray(
        x2.reshape(NBLK, T_BLK, KO, P).transpose(0, 3, 2, 1)
    )
    in_maps = []
    for c in range(N_CORES):
        lo, hi = c * OUT_SH, (c + 1) * OUT_SH
        wTc = np.ascontiguousarray(weight[lo:hi, :].astype(np.float32, copy=False).T)
        scc = np.ascontiguousarray(
            np.broadcast_to(weight_scale[lo:hi].astype(np.float32, copy=False)[None, :], (P, OUT_SH))
        )
        bic = np.ascontiguousarray(
            np.broadcast_to(bias[lo:hi].astype(np.float32, copy=False)[None, :], (P, OUT_SH))
        )
        in_maps.append({"xq": xq, "wT": wTc, "scale_rep": scc, "bias_rep": bic})
    return in_maps


def kernel(x, weight, weight_scale, bias, _trace=False):
    if "nc" not in _cache:
        _cache["nc"] = _build_program()
    nc = _cache["nc"]
    in_maps = _prep_inputs(x, weight, weight_scale, bias)
    res = bass_utils.run_bass_kernel_spmd(
        nc, in_maps, core_ids=list(range(N_CORES)), trace=_trace
    )
    _cache["last_result"] = res
    out = np.concatenate([res.results[c]["y"] for c in range(N_CORES)], axis=1)
    return out.reshape(B, S, OUT)
